# revision 24
# baseline (speedup 1.0000x reference)
"""ClusterMemory teacher loss kernel for 8x Trainium2 NeuronCores.

Strategy (tensor-parallel over the cluster/num_samples axis, per the
sharding hint): each of the 8 cores holds a 1024-row shard of each of the
three feature banks, computes A = -2 * x_hat @ f_shard^T on the tensor
engine (scales folded into the fp8 operands, DoubleRow perf mode), and
reduces each [128, NSH] tile to per-batch-row partials:

  L1 = sum_j exp(20 * s)     (CE#1 logsumexp partial, from the Exp
                              activation's row accumulator)
  U1 = sum_j u_j,  u = exp(d)  evaluated as the nested-square polynomial
       u = (PC*(z+PB)^2 + PD)^2 = v^2,  z = d^2 = x2 + f2 - 2s

All inputs are DMA'd in a partition-major layout ([128, KT, cols]) so every
descriptor is a contiguous >=4 KB line and the banks stream at HBM rate.
With L2-normalized bank rows f2 == 1 (+-2e-7), so z = q*A + (x2+1) and the
whole epilogue needs no [128, N] broadcast tensors - just a per-partition
bias column x2+1+PB.

The epilogue work is split across ACT and DVE so neither exceeds the
matmul rate; the two m-tiles of each branch use different (algebraically
equivalent) reduction schemes:
  m=0: ACT: Exp(accum->L1), Square(bias) -> s1; DVE: v = PC*s1+PD,
       bn_stats(v) -> (sum v, sum v^2) moments  [U1 = sum v^2]
  m=1: ACT: Exp(accum->L1) only; DVE: w = q*A + bias, w2 = w*w,
       bn_stats(w2) -> (sum w^2, sum w^4); host reconstructs
       U1 = PC^2*sum w^4 + 2*PC*PD*sum w^2 + PD^2*n
The final (br2, m1) tile runs its matmuls j-outer into two 512-wide psum
tiles so the j=0 epilogue overlaps the j=1 matmuls, shortening the tail.

Host (fp64) combine: CE1 = mean_b [log(sum_c L1) - 20*s_t];
CE2 = log(N+1) - mean_b [u_t / E], E = sum_c U1 (the exact CE2 has an
extra +U2/(2E^2) inside the log; that term is ~8e-9 relative -> dropped).
s_t, u_t are evaluated on the host with the same u-polynomial as the
device, so the polynomial's ~1e-4 error cancels in the softmax.
No collectives are needed; per-core outputs are 21 KiB."""

import numpy as np
import ml_dtypes

import concourse.bass as bass
import concourse.mybir as mybir
import concourse.tile as tile
from concourse import bacc
from concourse.bass_utils import run_bass_kernel_spmd

B = 256          # batch
D = 2048         # feature dim
N = 8192         # cluster count (total)
NCORES = 8
NSH = N // NCORES  # 1024 cluster rows per core
KT = D // 128      # 16 contraction chunks
MT = B // 128      # 2 partition tiles of the batch
JT = NSH // 512    # 2 matmul free-dim chunks
TEMP = 0.05
EPS = 1e-12
LAMBDA2 = 0.5
SCOL = 14  # stats columns per branch: [L1_a, L1_b, bn_j0(6), bn_j1(6)]

# u = exp(sqrt(z)) on z in [1.55, 2.45] as a nested-square polynomial
#   u = (PC*(z + PB)^2 + PD)^2     (max rel err 1.2e-4, which cancels to
# <1e-7 in the softmax-CE because the same polynomial is used for the
# host-side target term and softmax is invariant to common distortion).
PB = np.float32(-15.160572726694888)
PC = np.float32(-0.013651339885605563)
PD = np.float32(4.392563556355194)


def _poly_u(z):
    """Same u-polynomial as the device epilogue (fp64 on fp32 consts)."""
    r = float(PC) * (z + float(PB)) ** 2 + float(PD)
    return r * r

F32 = mybir.dt.float32

# mm dtype config: (mybir dtype, numpy dtype, range prescale)
_MM_CONFIGS = {
    "bf16": (mybir.dt.bfloat16, ml_dtypes.bfloat16, 1.0),
    "fp8": (mybir.dt.float8e4, ml_dtypes.float8_e4m3, 8.0),
}
import os as _os
MM_MODE = _os.environ.get("KMM_MODE", "fp8")

_cache = {}


class _only_combined_act_set:
    """Restrict the activation-table chooser to `natural_log_exp_and_others`
    during our compile: the greedy first-match chooser would otherwise bounce
    between tables (one ~2.7us table load per switch)."""

    def __enter__(self):
        self._orig = bacc.get_activation_tables
        orig = self._orig

        def patched(arch):
            tables = orig(arch)
            return {
                name: (funcs if name == "natural_log_exp_and_others" else set())
                for name, funcs in tables.items()
            }

        bacc.get_activation_tables = patched
        return self

    def __exit__(self, *exc):
        bacc.get_activation_tables = self._orig
        return False


def _build_nc(mode):
    mm_dt, _, sc = _MM_CONFIGS[mode]
    q = 1.0 / (sc * sc)  # descale for the psum values
    AF = mybir.ActivationFunctionType
    use_dr = mode == "fp8"
    kstep = 2 if use_dr else 1
    perf_mode = mybir.MatmulPerfMode.DoubleRow if use_dr else None
    KQ = 8           # ft half-chunks: 1 MB DMAs, 8 KB descriptors
    NCH = KT // KQ

    nc = bacc.Bacc(
        "TRN2",
        target_bir_lowering=False,
        debug=False,
        enable_asserts=False,
        num_devices=NCORES,
    )

    xt = nc.dram_tensor("xt", [3, 128, KT, B], mm_dt, kind="ExternalInput")
    ft = nc.dram_tensor("ft", [3, 128, KT, NSH], mm_dt, kind="ExternalInput")
    x2 = nc.dram_tensor("x2", [128, 3 * MT], F32, kind="ExternalInput")
    stats = nc.dram_tensor("stats", [MT, 128, 3 * SCOL], F32, kind="ExternalOutput")

    with tile.TileContext(nc) as tc:
        with (
            tc.tile_pool(name="xtp", bufs=2) as xt_pool,
            tc.tile_pool(name="ftp", bufs=4) as ft_pool,
            tc.tile_pool(name="x2p", bufs=1) as x2_pool,
            tc.tile_pool(name="scra", bufs=6) as scra_pool,
            tc.tile_pool(name="scrb", bufs=8) as scrb_pool,
            tc.tile_pool(name="stp", bufs=1) as st_pool,
            tc.tile_pool(name="psa", bufs=3, space="PSUM") as psa_pool,
            tc.tile_pool(name="psb", bufs=2, space="PSUM") as psb_pool,
        ):
            stats_sb = []
            for m in range(MT):
                st_t = st_pool.tile([128, 3 * SCOL], F32, name=f"st{m}", tag=f"st{m}")
                # zero-fill: scheme A rows leave the L1_b column unwritten
                nc.vector.memset(st_t, 0.0)
                stats_sb.append(st_t)

            # per-row x2 scalars in one small DMA up front
            x2t = x2_pool.tile([128, 3 * MT], F32, name="x2t", tag="x2t")
            nc.sync.dma_start(out=x2t, in_=x2[:, :])
            # per-partition bias column: x2 + f2(=1) + PB
            bias6 = x2_pool.tile([128, 3 * MT], F32, name="bias6", tag="bias6")
            nc.vector.tensor_scalar(
                out=bias6, in0=x2t, scalar1=float(1.0 + float(PB)), scalar2=None,
                op0=mybir.AluOpType.add,
            )

            def epi_m0(ps, st_t, br):
                """Full-width scheme A: ACT Exp+Square, DVE v + bn(v)."""
                c0 = SCOL * br
                col = 2 * br
                junk = scra_pool.tile([128, NSH], F32,
                                      name=f"junkA_{br}", tag="junkA")
                nc.scalar.activation(
                    junk, ps, AF.Exp, scale=-10.0 * q,
                    accum_out=st_t[:, c0:c0 + 1],
                )
                s1 = scra_pool.tile([128, NSH], F32, name=f"s1_{br}", tag="s1")
                nc.scalar.activation(s1, ps, AF.Square, scale=q,
                                     bias=bias6[:, col:col + 1])
                v = scra_pool.tile([128, NSH], F32, name=f"v_{br}", tag="v")
                nc.vector.tensor_scalar(
                    out=v, in0=s1, scalar1=float(PC), scalar2=float(PD),
                    op0=mybir.AluOpType.mult, op1=mybir.AluOpType.add,
                )
                for j in range(JT):
                    nc.vector.bn_stats(
                        out=st_t[:, c0 + 2 + 6 * j:c0 + 8 + 6 * j],
                        in_=v[:, 512 * j:512 * (j + 1)],
                    )

            def epi_m1_half(ps_h, st_t, br, j):
                """Half-width scheme B: ACT Exp only; DVE w, w^2, bn(w^2)."""
                c0 = SCOL * br
                col = 2 * br + 1
                junk = scrb_pool.tile([128, 512], F32,
                                      name=f"junkB_{br}_{j}", tag="junkB")
                nc.scalar.activation(
                    junk, ps_h, AF.Exp, scale=-10.0 * q,
                    accum_out=st_t[:, c0 + j:c0 + j + 1],
                )
                w = scrb_pool.tile([128, 512], F32, name=f"w_{br}_{j}", tag="w")
                nc.vector.tensor_scalar(
                    out=w, in0=ps_h, scalar1=q, scalar2=bias6[:, col:col + 1],
                    op0=mybir.AluOpType.mult, op1=mybir.AluOpType.add,
                )
                w2 = scrb_pool.tile([128, 512], F32, name=f"w2_{br}_{j}", tag="w2")
                nc.vector.tensor_tensor(out=w2, in0=w, in1=w,
                                        op=mybir.AluOpType.mult)
                nc.vector.bn_stats(
                    out=st_t[:, c0 + 2 + 6 * j:c0 + 8 + 6 * j], in_=w2,
                )

            for br in range(3):
                fks = []
                for h in range(NCH):
                    fk = ft_pool.tile([128, KQ, NSH], mm_dt,
                                      name=f"fk_{br}_{h}", tag="fk")
                    nc.sync.dma_start(
                        out=fk, in_=ft[br, :, h * KQ:(h + 1) * KQ, :])
                    fks.append(fk)
                    if h == 0:
                        xk = xt_pool.tile([128, KT, B], mm_dt,
                                          name=f"xk_{br}", tag="xk")
                        nc.sync.dma_start(out=xk, in_=xt[br])

                # m-outer so m=0's epilogue overlaps m=1's matmuls; only the
                # last m-tile's epilogue is exposed at the kernel tail.
                for m in range(MT):
                    last = (br == 2 and m == MT - 1)

                    def mm(out_ap, j, k):
                        fk = fks[k // KQ]
                        kk = k % KQ
                        if use_dr:
                            lhsT = xk[:, k:k + 2, m * 128:(m + 1) * 128]
                            rhs = fk[:, kk:kk + 2, j * 512:(j + 1) * 512]
                        else:
                            lhsT = xk[:, k, m * 128:(m + 1) * 128]
                            rhs = fk[:, kk, j * 512:(j + 1) * 512]
                        nc.tensor.matmul(
                            out_ap, lhsT, rhs,
                            start=(k == 0), stop=(k == KT - kstep),
                            perf_mode=perf_mode,
                        )

                    if m == 0:
                        ps = psa_pool.tile([128, NSH], F32,
                                           name=f"ps_{br}_0", tag="psa")
                        for k in range(0, KT, kstep):
                            for j in range(JT):
                                mm(ps[:, j * 512:(j + 1) * 512], j, k)
                        epi_m0(ps, stats_sb[0], br)
                    elif not last:
                        ps = psa_pool.tile([128, NSH], F32,
                                           name=f"ps_{br}_1", tag="psa")
                        for k in range(0, KT, kstep):
                            for j in range(JT):
                                mm(ps[:, j * 512:(j + 1) * 512], j, k)
                        for j in range(JT):
                            epi_m1_half(ps[:, j * 512:(j + 1) * 512],
                                        stats_sb[1], br, j)
                    else:
                        # j-outer on the final tile, separate 1-bank psum
                        # tiles: the j=0 epilogue overlaps the j=1 matmuls
                        for j in range(JT):
                            psj = psb_pool.tile([128, 512], F32,
                                                name=f"ps_l_{j}", tag="psb")
                            for k in range(0, KT, kstep):
                                mm(psj[:, :], j, k)
                            epi_m1_half(psj, stats_sb[1], br, j)

            for m in range(MT):
                nc.gpsimd.dma_start(out=stats[m], in_=stats_sb[m])

    with _only_combined_act_set():
        nc.compile()
    return nc


def _get_nc(mode):
    if mode not in _cache:
        _cache[mode] = _build_nc(mode)
    return _cache[mode]


def _prepare_branch(x_raw, f, mode):
    """Host-side prep for one branch. Returns per-core input arrays and the
    fp64 host-side quantities."""
    _, np_dt, sc = _MM_CONFIGS[mode]
    x_raw = np.asarray(x_raw, dtype=np.float32)
    f = np.asarray(f, dtype=np.float32)

    n = np.sqrt(np.sum(x_raw.astype(np.float64) ** 2, axis=1, keepdims=True))
    xh64 = x_raw.astype(np.float64) / np.maximum(n, EPS)
    xh = xh64.astype(np.float32)

    x2 = np.sum(xh.astype(np.float64) ** 2, axis=1)   # [B], ~1.0

    # partition-major [128, KT, cols]: contiguous per-partition lines
    xt = ((-2.0 * sc) * xh.T).astype(np_dt)                       # [D, B]
    xt = np.ascontiguousarray(xt.reshape(KT, 128, B).transpose(1, 0, 2))
    fT = (sc * f.T).astype(np_dt)                                 # [D, N]
    ft_shards = [
        np.ascontiguousarray(
            fT[:, c * NSH:(c + 1) * NSH].reshape(KT, 128, NSH).transpose(1, 0, 2))
        for c in range(NCORES)
    ]
    x2_dev = x2.astype(np.float32).reshape(MT, 128).T  # [128, MT]
    return xt, ft_shards, x2_dev, xh, x2


def _bn_moments(st, base):
    """bn_stats 6-col block at `base`: two (cnt, mean, cnt*var) groups.
    Returns (sum x, sum x^2, cnt) accumulated over both groups."""
    s1 = np.zeros(st.shape[:-1])
    s2 = np.zeros(st.shape[:-1])
    cn = np.zeros(st.shape[:-1])
    for off in (0, 3):
        c = st[..., base + off]
        mn = st[..., base + off + 1]
        cv = st[..., base + off + 2]
        s1 = s1 + c * mn
        s2 = s2 + cv + c * mn * mn
        cn = cn + c
    return s1, s2, cn


def _host_combine(stats_by_core, xh, x2, f, targets):
    """stats_by_core: [NCORES] of [MT, 128, SCOL] for this branch.
    Returns the branch loss (fp64)."""
    st = np.stack(stats_by_core).astype(np.float64)  # [NC, MT, 128, SCOL]

    # m=0 rows (scheme A): L1 in col0; U1 = sum v^2 from bn(v)
    st0 = st[:, 0]
    L1_m0 = st0[..., 0].sum(axis=0)
    U1_m0 = np.zeros_like(L1_m0)
    for base in (2, 8):
        _, s2, _ = _bn_moments(st0, base)
        U1_m0 = U1_m0 + s2.sum(axis=0)

    # m=1 rows (scheme B): L1 in cols 0+1; bn(w2) gives sum w^2, sum w^4;
    # U1 = PC^2*sum w^4 + 2*PC*PD*sum w^2 + PD^2*n
    st1 = st[:, 1]
    L1_m1 = (st1[..., 0] + st1[..., 1]).sum(axis=0)
    Sw2 = np.zeros_like(L1_m1)
    Sw4 = np.zeros_like(L1_m1)
    cnt = np.zeros_like(L1_m1)
    for base in (2, 8):
        s1, s2, cn = _bn_moments(st1, base)
        Sw2 = Sw2 + s1.sum(axis=0)
        Sw4 = Sw4 + s2.sum(axis=0)
        cnt = cnt + cn.sum(axis=0)
    pc, pd = float(PC), float(PD)
    U1_m1 = pc * pc * Sw4 + 2.0 * pc * pd * Sw2 + pd * pd * cnt

    L1 = np.concatenate([L1_m0, L1_m1])   # [B] (row = m*128 + p)
    E = np.concatenate([U1_m0, U1_m1])

    f_t = np.asarray(f, np.float32)[targets].astype(np.float64)   # [B, D]
    s_t = np.sum(xh.astype(np.float64) * f_t, axis=1)
    # f2 == 1 for the L2-normalized banks (same assumption as the device)
    z_t = np.maximum(x2 + 1.0 - 2.0 * s_t, 0.0)
    u_t = _poly_u(z_t)  # same polynomial as the device (softmax-consistent)

    ce1 = np.mean(np.log(L1) - s_t / TEMP)
    # exact: log(N + 1 + U2/(2E^2)); the U2 term is ~8e-9 relative -> drop
    ce2 = np.log(N + 1.0) - np.mean(u_t / E)
    return ce1 + ce2


def run(inputs, inputs_up, inputs_down, targets, epoch, features, features_up,
        features_down, trace=False):
    mode = MM_MODE
    nc = _get_nc(mode)
    targets = np.asarray(targets).astype(np.int64)

    xs = [inputs, inputs_up, inputs_down]
    fs = [features, features_up, features_down]

    prep = [_prepare_branch(x, f, mode) for x, f in zip(xs, fs)]

    in_maps = []
    for c in range(NCORES):
        in_maps.append({
            "xt": np.stack([p[0] for p in prep]),                 # [3,128,KT,B]
            "ft": np.stack([p[1][c] for p in prep]),              # [3,128,KT,NSH]
            # [128, 3*MT]: column 2*br+m holds x2 of batch rows m*128..m*128+127
            "x2": np.ascontiguousarray(
                np.concatenate([p[2] for p in prep], axis=1)),
        })

    res = run_bass_kernel_spmd(nc, in_maps, list(range(NCORES)), trace=trace)

    branch_losses = []
    for bi in range(3):
        stats_by_core = [
            res.results[c]["stats"][:, :, SCOL * bi:SCOL * (bi + 1)]
            for c in range(NCORES)
        ]
        _, _, _, xh, x2 = prep[bi]
        branch_losses.append(
            _host_combine(stats_by_core, xh, x2,
                          np.asarray(fs[bi], np.float32), targets)
        )

    l_mid, l_up, l_down = branch_losses
    loss = (1.0 - LAMBDA2) * l_mid + LAMBDA2 * (l_up + l_down)
    out = np.float32(loss)
    return (out, res) if trace else out


def kernel(**inputs):
    return run(**inputs)


# revision 25
# speedup vs baseline: 1.1622x; 1.1622x over previous
"""ClusterMemory teacher loss kernel for 8x Trainium2 NeuronCores.

Strategy (tensor-parallel over the cluster/num_samples axis, per the
sharding hint): each of the 8 cores holds a 1024-row shard of each of the
three feature banks, computes A = -2 * x_hat @ f_shard^T on the tensor
engine (scales folded into the fp8 operands, DoubleRow perf mode), and
reduces each [128, NSH] psum tile to one partial per batch row:

  L1 = sum_j exp(20 * s)     (CE#1 logsumexp partial, from the Exp
                              activation's row accumulator)

All inputs are DMA'd in a partition-major layout ([128, KT, cols]) so
every descriptor is a contiguous >=2 KB line and the banks stream at
HBM rate.  The k-chunks are consumed m-interleaved (both batch tiles per
arriving chunk) and the chunk sizes taper at the start of branch 0 (fast
matmul spin-up) and the end of branch 2 (minimal post-DMA work).

Host (fp64) combine:
  CE1 = mean_b [log(sum_c L1) - 20*s_t]
  CE2 = log(N+1) exactly: the exact term is
        mean_b[log(N + 1 + U2/(2E^2)) - u_t/E] with u = exp(d) and
        E = sum_j u_j; U2/(2E^2·(N+1)) ~ 8e-9 and u_t/E ~ 1.2e-4, i.e.
        ~7e-6 relative on the final loss - two orders below the fp8
        matmul noise and three below the 2e-2 gate - so both are dropped
        and CE2 needs no device work at all.
No collectives; per-core output is 3 KiB of L1 partials."""

import numpy as np
import ml_dtypes

import concourse.bass as bass
import concourse.mybir as mybir
import concourse.tile as tile
from concourse import bacc
from concourse.bass_utils import run_bass_kernel_spmd

B = 256          # batch
D = 2048         # feature dim
N = 8192         # cluster count (total)
NCORES = 8
NSH = N // NCORES  # 1024 cluster rows per core
KT = D // 128      # 16 contraction chunks
MT = B // 128      # 2 partition tiles of the batch
JT = NSH // 512    # 2 matmul free-dim chunks
TEMP = 0.05
EPS = 1e-12
LAMBDA2 = 0.5

F32 = mybir.dt.float32

# mm dtype config: (mybir dtype, numpy dtype, range prescale)
_MM_CONFIGS = {
    "bf16": (mybir.dt.bfloat16, ml_dtypes.bfloat16, 1.0),
    "fp8": (mybir.dt.float8e4, ml_dtypes.float8_e4m3, 8.0),
}
import os as _os
MM_MODE = _os.environ.get("KMM_MODE", "fp8")

# ft k-chunk split per branch (in units of 128-deep k-slices):
# small first chunks let the matmul stream start early; small last
# chunks leave almost no matmul work after the final DMA byte lands.
_CHUNKS = [
    [2, 2, 4, 8],     # branch 0
    [8, 8],           # branch 1
    [8, 4, 2, 2],     # branch 2
]

_cache = {}


class _only_combined_act_set:
    """Restrict the activation-table chooser to `natural_log_exp_and_others`
    during our compile so only one ~2.7us activation table load happens."""

    def __enter__(self):
        self._orig = bacc.get_activation_tables
        orig = self._orig

        def patched(arch):
            tables = orig(arch)
            return {
                name: (funcs if name == "natural_log_exp_and_others" else set())
                for name, funcs in tables.items()
            }

        bacc.get_activation_tables = patched
        return self

    def __exit__(self, *exc):
        bacc.get_activation_tables = self._orig
        return False


def _build_nc(mode):
    mm_dt, _, sc = _MM_CONFIGS[mode]
    q = 1.0 / (sc * sc)  # descale for the psum values
    AF = mybir.ActivationFunctionType
    use_dr = mode == "fp8"
    kstep = 2 if use_dr else 1
    perf_mode = mybir.MatmulPerfMode.DoubleRow if use_dr else None

    nc = bacc.Bacc(
        "TRN2",
        target_bir_lowering=False,
        debug=False,
        enable_asserts=False,
        num_devices=NCORES,
    )

    xt = nc.dram_tensor("xt", [3, 128, KT, B], mm_dt, kind="ExternalInput")
    ft = nc.dram_tensor("ft", [3, 128, KT, NSH], mm_dt, kind="ExternalInput")
    stats = nc.dram_tensor("stats", [MT, 128, 3], F32, kind="ExternalOutput")

    with tile.TileContext(nc) as tc:
        with (
            tc.tile_pool(name="xtp", bufs=2) as xt_pool,
            tc.tile_pool(name="ftp", bufs=5) as ft_pool,
            tc.tile_pool(name="scr", bufs=3) as scr_pool,
            tc.tile_pool(name="stp", bufs=1) as st_pool,
            tc.tile_pool(name="ps", bufs=4, space="PSUM") as psum_pool,
        ):
            stats_sb = []
            for m in range(MT):
                st_t = st_pool.tile([128, 3], F32, name=f"st{m}", tag=f"st{m}")
                stats_sb.append(st_t)

            for br in range(3):
                chunks = _CHUNKS[br]
                bounds = []
                k0 = 0
                for w in chunks:
                    bounds.append((k0, k0 + w))
                    k0 += w

                fks = {}
                for ci, (k0, k1) in enumerate(bounds):
                    fk = ft_pool.tile([128, k1 - k0, NSH], mm_dt,
                                      name=f"fk_{br}_{ci}", tag="fk")
                    nc.sync.dma_start(out=fk, in_=ft[br, :, k0:k1, :])
                    fks[ci] = fk
                    if ci == 0:
                        xk = xt_pool.tile([128, KT, B], mm_dt,
                                          name=f"xk_{br}", tag="xk")
                        nc.sync.dma_start(out=xk, in_=xt[br])

                pss = [
                    psum_pool.tile([128, NSH], F32, name=f"ps_{br}_{m}", tag="ps")
                    for m in range(MT)
                ]

                # consume each arriving chunk with BOTH m-tiles immediately:
                # after the branch's last DMA byte only that chunk's few
                # matmuls remain.
                for ci, (k0, k1) in enumerate(bounds):
                    fk = fks[ci]
                    for m in range(MT):
                        for k in range(k0, k1, kstep):
                            kk = k - k0
                            for j in range(JT):
                                if use_dr:
                                    lhsT = xk[:, k:k + 2, m * 128:(m + 1) * 128]
                                    rhs = fk[:, kk:kk + 2, j * 512:(j + 1) * 512]
                                else:
                                    lhsT = xk[:, k, m * 128:(m + 1) * 128]
                                    rhs = fk[:, kk, j * 512:(j + 1) * 512]
                                nc.tensor.matmul(
                                    pss[m][:, j * 512:(j + 1) * 512], lhsT, rhs,
                                    start=(k == 0), stop=(k == KT - kstep),
                                    perf_mode=perf_mode,
                                )

                # L1 partial: sum_j exp(20 s) = sum_j exp(-10 * q * A)
                for m in range(MT):
                    junk = scr_pool.tile([128, NSH], F32,
                                         name=f"junk_{br}_{m}", tag="junk")
                    nc.scalar.activation(
                        junk, pss[m], AF.Exp, scale=-10.0 * q,
                        accum_out=stats_sb[m][:, br:br + 1],
                    )

            for m in range(MT):
                nc.gpsimd.dma_start(out=stats[m], in_=stats_sb[m])

    with _only_combined_act_set():
        nc.compile()
    return nc


def _get_nc(mode):
    if mode not in _cache:
        _cache[mode] = _build_nc(mode)
    return _cache[mode]


def _prepare_branch(x_raw, f, mode):
    """Host-side prep for one branch. Returns per-core input arrays and the
    fp64 host-side quantities."""
    _, np_dt, sc = _MM_CONFIGS[mode]
    x_raw = np.asarray(x_raw, dtype=np.float32)
    f = np.asarray(f, dtype=np.float32)

    n = np.sqrt(np.sum(x_raw.astype(np.float64) ** 2, axis=1, keepdims=True))
    xh64 = x_raw.astype(np.float64) / np.maximum(n, EPS)
    xh = xh64.astype(np.float32)

    # partition-major [128, KT, cols]: contiguous per-partition lines
    xt = ((-2.0 * sc) * xh.T).astype(np_dt)                       # [D, B]
    xt = np.ascontiguousarray(xt.reshape(KT, 128, B).transpose(1, 0, 2))
    fT = (sc * f.T).astype(np_dt)                                 # [D, N]
    ft_shards = [
        np.ascontiguousarray(
            fT[:, c * NSH:(c + 1) * NSH].reshape(KT, 128, NSH).transpose(1, 0, 2))
        for c in range(NCORES)
    ]
    return xt, ft_shards, xh


def _host_combine(stats_by_core, xh, f, targets):
    """stats_by_core: [NCORES] of [MT, 128] L1 partials for this branch.
    Returns the branch loss (fp64)."""
    st = np.stack([s.reshape(B) for s in stats_by_core]).astype(np.float64)
    L1 = st.sum(axis=0)   # [B]

    f_t = np.asarray(f, np.float32)[targets].astype(np.float64)   # [B, D]
    s_t = np.sum(xh.astype(np.float64) * f_t, axis=1)

    ce1 = np.mean(np.log(L1) - s_t / TEMP)
    # CE2 = log(N + 1 + U2/(2E^2)) - mean(u_t/E); the U2 term is ~8e-9 and
    # u_t/E ~ 1.2e-4 (7e-6 relative on the loss) -> log(N+1) exactly.
    ce2 = np.log(N + 1.0)
    return ce1 + ce2


def run(inputs, inputs_up, inputs_down, targets, epoch, features, features_up,
        features_down, trace=False):
    mode = MM_MODE
    nc = _get_nc(mode)
    targets = np.asarray(targets).astype(np.int64)

    xs = [inputs, inputs_up, inputs_down]
    fs = [features, features_up, features_down]

    prep = [_prepare_branch(x, f, mode) for x, f in zip(xs, fs)]

    in_maps = []
    for c in range(NCORES):
        in_maps.append({
            "xt": np.stack([p[0] for p in prep]),                 # [3,128,KT,B]
            "ft": np.stack([p[1][c] for p in prep]),              # [3,128,KT,NSH]
        })

    res = run_bass_kernel_spmd(nc, in_maps, list(range(NCORES)), trace=trace)

    branch_losses = []
    for bi in range(3):
        stats_by_core = [res.results[c]["stats"][:, :, bi] for c in range(NCORES)]
        _, _, xh = prep[bi]
        branch_losses.append(
            _host_combine(stats_by_core, xh,
                          np.asarray(fs[bi], np.float32), targets)
        )

    l_mid, l_up, l_down = branch_losses
    loss = (1.0 - LAMBDA2) * l_mid + LAMBDA2 * (l_up + l_down)
    out = np.float32(loss)
    return (out, res) if trace else out


def kernel(**inputs):
    return run(**inputs)
